# revision 12
# baseline (speedup 1.0000x reference)
"""Masked attention kernel for Trainium2, SPMD over 8 NeuronCores.

Problem: nn_AttentionModule (N=16 heads, A=B=2048, H=64, fp32, bool key mask).
Sharding: 2 heads per core (data/head parallel, no cross-core comms).

Per-core algorithm (2 heads, packed):
  S^T[b,a] = K[b,:] . Q[a,:]         (PE, bf16 operands, heads packed in PE rows 0-63 / 64-127)
  P^T      = exp(S^T * 1/sqrt(H))    (ACT, exact; mask applied via zeroed V''-rows, not here)
  CtxT/den = V''^T @ P^T             (PE; V'' = [V * m | m], m = 1-mask -> row 64 = denominator)
  out      = transpose(CtxT) * 1/den (PE transpose + DVE reciprocal + tensor_scalar)

Host side only reshapes/permutes inputs (sharding/layout prep: per-head key
compaction drops fully-masked key tiles; mask is still applied on-device via
the V'' mask column) and concatenates the 8 per-core outputs.
"""

import numpy as np

N_HEADS, A_FULL, B_FULL, H_DIM = 16, 2048, 2048, 64
N_CORES = 8
HPC = N_HEADS // N_CORES  # 2 heads per core

_BUILD_CACHE = {}


def build_nc(A=A_FULL, B=B_FULL, H=H_DIM, CHUNK=512, NJ=None, dve_js=()):
    """Build the SPMD Bass program for one core (2 heads)."""
    import concourse.bacc as bacc
    import concourse.tile as tile
    from concourse import mybir
    from concourse.masks import make_identity

    f32 = mybir.dt.float32
    bf16 = mybir.dt.bfloat16
    Exp = mybir.ActivationFunctionType.Exp

    if NJ is None:
        NJ = B // 128   # key tiles per head (after host-side compaction)
    B = NJ * 128
    NCH = A // CHUNK    # query chunks per head
    NT = CHUNK // 128   # 128-row transposes per chunk
    SCALE = 1.0 / float(np.sqrt(H))

    nc = bacc.Bacc()

    qT = nc.declare_dram_parameter("qT", [HPC, H, A], f32, isOutput=False)
    kT = nc.declare_dram_parameter("kT", [HPC, H, B], f32, isOutput=False)
    v = nc.declare_dram_parameter("v", [HPC, B, H], f32, isOutput=False)
    m01 = nc.declare_dram_parameter("m01", [128, HPC * NJ], f32, isOutput=False)
    out = nc.declare_dram_parameter("out", [HPC, A, H], f32, isOutput=True)

    qT_flat = qT.rearrange("h d a -> (h d) a")  # [128, A]
    kT_flat = kT.rearrange("h d b -> (h d) b")  # [128, B]

    with tile.TileContext(nc) as tc:
        import contextlib

        with contextlib.ExitStack() as ctx:
            const = ctx.enter_context(tc.tile_pool(name="const", bufs=1))
            ptp = ctx.enter_context(tc.tile_pool(name="ptp", bufs=2))
            outp = ctx.enter_context(tc.tile_pool(name="outp", bufs=3))
            stp = ctx.enter_context(tc.tile_pool(name="stp", bufs=2, space="PSUM"))
            otp = ctx.enter_context(tc.tile_pool(name="otp", bufs=2, space="PSUM"))
            tpp = ctx.enter_context(tc.tile_pool(name="tpp", bufs=2, space="PSUM"))

            # ---- constants / inputs ----
            warm = const.tile([128, 1], f32, name="warm")
            nc.vector.memset(warm, 0.0)
            nc.scalar.activation(warm, warm, Exp, scale=1.0)

            ident = const.tile([128, 128], f32)
            make_identity(nc, ident)

            m01_sb = const.tile([128, HPC * NJ], f32)
            nc.sync.dma_start(out=m01_sb, in_=m01[:, :])

            kt_sb = const.tile([128, B], bf16)
            nc.gpsimd.dma_start(out=kt_sb, in_=kT_flat[:, :])

            qt_sb = const.tile([128, A], bf16)
            nc.gpsimd.dma_start(out=qt_sb, in_=qT_flat[:, :])

            # V'' = [V * m | m]; built from raw V + ones col, masked on GPSIMD
            vvr = const.tile([128, HPC, NJ, H], bf16)
            vv = const.tile([128, HPC, NJ, H + 1], bf16)
            for h in range(HPC):
                nc.gpsimd.dma_start(
                    out=vvr[:, h, :, :],
                    in_=v[h].rearrange("(j p) d -> p j d", p=128),
                )
            # mask column: vv[..., H] = m01 (denominator row of V'')
            nc.gpsimd.tensor_copy(
                vv[:, :, :, H], m01_sb[:, :].rearrange("p (h j) -> p h j", h=HPC)
            )
            for h in range(HPC):
                for j in range(NJ):
                    nc.gpsimd.tensor_scalar_mul(
                        vv[:, h, j, 0:H],
                        vvr[:, h, j, :],
                        m01_sb[:, h * NJ + j : h * NJ + j + 1],
                    )

            # ---- main pipeline ----
            pt_tiles = {}
            ot_tiles = {}

            for c in range(NCH + 1):
                do_mm1 = c < NCH
                cm = c - 1

                if do_mm1:
                    pt_tiles[c] = ptp.tile([128, HPC, NJ, CHUNK], bf16, tag="pt", name="pt")
                if cm >= 0:
                    ot_tiles[cm] = [
                        otp.tile([H + 1, CHUNK], f32, tag="ot", name="ot") for _ in range(HPC)
                    ]

                for j in range(NJ):
                    if do_mm1:
                        BANK = max(CHUNK, 512)
                        stf = stp.tile([128, HPC, BANK], f32, tag="st", name="st")
                        st = stf[:, :, 0:CHUNK]
                        for h in range(HPC):
                            nc.tensor.matmul(
                                st[:, h, :],
                                lhsT=kt_sb[
                                    64 * h : 64 * (h + 1), j * 128 : (j + 1) * 128
                                ],
                                rhs=qt_sb[
                                    64 * h : 64 * (h + 1),
                                    c * CHUNK : (c + 1) * CHUNK,
                                ],
                                start=True,
                                stop=True,
                                tile_position=(64 * h, 0),
                            )
                        pt = pt_tiles[c]
                        if j in dve_js:
                            # Schraudolph exp on DVE (approximate)
                            EXP_A = float(SCALE * (2 ** 7) / np.log(2.0))
                            EXP_B = float(127.0 * (2 ** 7) - 1.43)
                            pt_i = pt.bitcast(mybir.dt.int16)
                            nc.vector.tensor_scalar(
                                pt_i[:, :, j, :],
                                st[:, :, :],
                                EXP_A,
                                EXP_B,
                                op0=mybir.AluOpType.mult,
                                op1=mybir.AluOpType.add,
                            )
                        else:
                            nc.scalar.activation(
                                pt[:, :, j, :], st[:, :, :], Exp, scale=SCALE
                            )

                    if cm >= 0:
                        ptm = pt_tiles[cm]
                        for h in range(HPC):
                            nc.tensor.matmul(
                                ot_tiles[cm][h][:, :],
                                lhsT=vv[:, h, j, :],
                                rhs=ptm[:, h, j, :],
                                start=(j == 0),
                                stop=(j == NJ - 1),
                            )

                if cm >= 0:
                    # post-process chunk cm: transpose, normalize, store
                    for h in range(HPC):
                        ot_sb = outp.tile([H + 1, CHUNK], f32, tag="otsb", name="otsb")
                        nc.vector.tensor_copy(ot_sb, ot_tiles[cm][h][:, :])
                        tp = tpp.tile([128, NT, H + 1], f32, tag="tp", name="tp")
                        for t in range(NT):
                            nc.tensor.transpose(
                                tp[:, t, :],
                                ot_sb[:, t * 128 : (t + 1) * 128],
                                ident[0 : H + 1, 0 : H + 1],
                            )
                        rc = outp.tile([128, NT], f32, tag="rc", name="rc")
                        nc.vector.reciprocal(rc, tp[:, :, H])
                        fo = outp.tile([128, NT, H], f32, tag="fo", name="fo")
                        for t in range(NT):
                            nc.vector.tensor_scalar_mul(
                                fo[:, t, :], tp[:, t, 0:H], rc[:, t : t + 1]
                            )
                        nc.sync.dma_start(
                            out=out[h][
                                cm * CHUNK : (cm + 1) * CHUNK, :
                            ].rearrange("(t p) d -> p t d", p=128),
                            in_=fo,
                        )
    nc.compile()
    return nc


def _get_nc(key):
    if key not in _BUILD_CACHE:
        A, B, H, CHUNK, NJ, dve_js = key
        _BUILD_CACHE[key] = build_nc(A, B, H, CHUNK, NJ, dve_js)
    return _BUILD_CACHE[key]


def compact_nj(mask):
    """Number of 128-key tiles needed per head after masked-key compaction."""
    mask = np.asarray(mask)
    nu = (~mask).sum(axis=1).max()
    return max(1, int(-(-int(nu) // 128)))


def make_in_maps(query, key, value, mask, hpc=HPC, nj=None):
    """Shard + lay out full inputs into per-core input maps.

    Keys/values are compacted per head: a stable permutation puts unmasked
    keys first, and only the first nj*128 keys are shipped. Padded slots get
    zero K/V and m01=0, so the on-device mask column still kills them.
    """
    query = np.asarray(query, dtype=np.float32)
    key = np.asarray(key, dtype=np.float32)
    value = np.asarray(value, dtype=np.float32)
    mask = np.asarray(mask)
    n, b = mask.shape
    if nj is None:
        nj = compact_nj(mask)
    bc = nj * 128
    in_maps = []
    for core in range(n // hpc):
        h0 = core * hpc
        qT = np.ascontiguousarray(query[h0 : h0 + hpc].transpose(0, 2, 1))
        kc = np.zeros((hpc, bc, query.shape[2]), np.float32)
        vc = np.zeros((hpc, bc, query.shape[2]), np.float32)
        m01f = np.zeros((hpc, bc), np.float32)
        for h in range(hpc):
            keep = np.flatnonzero(~mask[h0 + h])
            nk = min(len(keep), bc)
            kc[h, :nk] = key[h0 + h, keep[:nk]]
            vc[h, :nk] = value[h0 + h, keep[:nk]]
            m01f[h, :nk] = 1.0
        kT = np.ascontiguousarray(kc.transpose(0, 2, 1))
        m01 = np.ascontiguousarray(
            m01f.reshape(hpc, nj, 128).transpose(2, 0, 1)
        ).reshape(128, hpc * nj)
        in_maps.append({"qT": qT, "kT": kT, "v": vc, "m01": m01})
    return in_maps


def _run(query, key, value, mask, trace=False):
    from concourse.bass_utils import run_bass_kernel_spmd

    query = np.asarray(query, dtype=np.float32)
    n, a, h = query.shape
    assert n == N_CORES * HPC, f"expected {N_CORES * HPC} heads, got {n}"
    nj = compact_nj(mask)
    nc = _get_nc((a, nj * 128, h, 512, nj, ()))
    in_maps = make_in_maps(query, key, value, mask, nj=nj)
    res = run_bass_kernel_spmd(nc, in_maps, list(range(N_CORES)), trace=trace)
    out = np.concatenate([res.results[i]["out"] for i in range(N_CORES)], axis=0)
    return np.ascontiguousarray(out.astype(np.float32)), res


def kernel(query, key, value, mask):
    out, _ = _run(query, key, value, mask, trace=False)
    return out


def kernel_profiled(query, key, value, mask):
    out, res = _run(query, key, value, mask, trace=True)
    return out, res

